# revision 1
# baseline (speedup 1.0000x reference)
"""AttnBlock v3: GN folded into QKV weights; bf16 attention core;
host-permuted x so one SPMD program serves all 8 cores.

Sharding: core = (batch b in {0,1}) x (query slice s in {0..3}, 1024
queries).  Each core redundantly computes full K / V^T for its batch
(avoids cross-core collectives), attention for its query slice only.
The host rolls x columns per core so the core's query block is always
columns 0:1024 -- identical program, per-core data.

Math: h = GN(x) = A_c * x + B_c per channel (A, B from runtime stats).
  q = (wq*A)@x + (wq@B + bq)     weight columns scaled on device
  k = (wk*A)@x   (k-bias shifts scores by a per-query constant ->
                  cancelled by softmax -> dropped)
  v = (wv*A)@x + const; attn rows sum to 1, so the v-bias passes through
      attention -> folded into the projection bias on device:
      bp_dev = bp + wp@bv + wp@(wv@B).
x is never normalized on device; production matmuls consume bf16 x.

Precision: stats fp32 (DVE reduce + ACT Square accum); scores/attn/denom
matmuls bf16 (fp32 PSUM accumulate); projection f32r; softmax without
max-subtraction (scaled scores are ~N(0,1)).
"""

import os
import sys

import numpy as np

for _p in ("/opt/trn_rl_repo", "/root/.axon_site/_ro/trn_rl_repo"):
    if os.path.isdir(_p) and _p not in sys.path:
        sys.path.insert(0, _p)

B, C, H, W = 2, 512, 64, 64
N = H * W
G = 32
GS = C // G
EPS = 1e-6
NCORES = 8
QS = N // 4               # 1024 queries per core
NHALF = 2                 # key halves
JQ = N // NHALF           # 2048 keys per half
JT = JQ // 128            # 16 key tiles per half
ICH = 512                 # query chunk
NCH = QS // ICH           # 2 chunks
CT = C // 128             # 4 channel tiles
SCALE = float(C) ** -0.5

_CACHE = {}


def _build():
    import contextlib

    import concourse.mybir as mybir
    import concourse.tile as tile
    from concourse import bacc
    from concourse.alu_op_type import AluOpType as alu

    f32 = mybir.dt.float32
    f32r = mybir.dt.float32r
    bf16 = mybir.dt.bfloat16
    AF = mybir.ActivationFunctionType
    AX = mybir.AxisListType

    nc = bacc.Bacc("TRN2", target_bir_lowering=False, debug=False,
                   num_devices=NCORES)

    xbf = nc.dram_tensor("xbf", [C, N], bf16, kind="ExternalInput").ap()
    xsf = nc.dram_tensor("xsf", [C, QS], f32, kind="ExternalInput").ap()
    m0T = nc.dram_tensor("m0T", [C, C], bf16, kind="ExternalInput").ap()
    qkbc = nc.dram_tensor("qkbc", [C, 1], f32, kind="ExternalInput").ap()
    wvT = nc.dram_tensor("wvT", [C, C], bf16, kind="ExternalInput").ap()
    wpT = nc.dram_tensor("wpT", [C, C], f32, kind="ExternalInput").ap()
    gamma = nc.dram_tensor("gamma", [C, 1], f32, kind="ExternalInput").ap()
    beta = nc.dram_tensor("beta", [C, 1], f32, kind="ExternalInput").ap()
    bqv = nc.dram_tensor("bq", [C, 1], f32, kind="ExternalInput").ap()
    bpv = nc.dram_tensor("bp", [C, 1], f32, kind="ExternalInput").ap()
    sel = nc.dram_tensor("sel", [128, 8], f32, kind="ExternalInput").ap()
    selT = nc.dram_tensor("selT", [8, 128], f32, kind="ExternalInput").ap()
    onesb = nc.dram_tensor("onesb", [128, 1], bf16, kind="ExternalInput").ap()
    out_d = nc.dram_tensor("out", [C, QS], f32, kind="ExternalOutput").ap()

    def mm(ps, lhsT, rhs, start, stop):
        nc.tensor.matmul(ps, lhsT, rhs, start=start, stop=stop)

    with tile.TileContext(nc) as tc:
        outer = contextlib.ExitStack()
        with outer:
            cpool = outer.enter_context(tc.tile_pool(name="const", bufs=1))
            x_p = outer.enter_context(tc.tile_pool(name="xbf", bufs=1))
            acc_p = outer.enter_context(tc.tile_pool(name="acc", bufs=1))
            w_p = outer.enter_context(tc.tile_pool(name="wts", bufs=1))
            q_p = outer.enter_context(tc.tile_pool(name="q", bufs=1))
            vT_p = outer.enter_context(tc.tile_pool(name="vT", bufs=JT))
            e_p = outer.enter_context(tc.tile_pool(name="expT", bufs=JT + 4))
            wp_p = outer.enter_context(tc.tile_pool(name="wp", bufs=1))
            xs_p = outer.enter_context(tc.tile_pool(name="xs", bufs=1))
            f_p = outer.enter_context(tc.tile_pool(name="fin", bufs=1))

            # ---- x first (stats critical path), then consts/weights ----
            # 1024-column chunks so bn_stats starts on the first chunk
            x_t = []
            for t in range(CT):
                row = []
                for c in range(N // 1024):
                    xt = x_p.tile([128, 1024], bf16, tag=f"x{t}_{c}",
                                  name=f"x{t}_{c}")
                    nc.sync.dma_start(
                        xt[:], xbf[t * 128:(t + 1) * 128,
                                   c * 1024:(c + 1) * 1024])
                    row.append(xt)
                x_t.append(row)

            def xsl(ci, start, size):
                c, off = divmod(start, 1024)
                assert off + size <= 1024
                return x_t[ci][c][:, off:off + size]
            sel_t = cpool.tile([128, 8], f32, tag="sel")
            nc.sync.dma_start(sel_t[:], sel[:])
            selT_t = cpool.tile([8, 128], f32, tag="selT")
            nc.sync.dma_start(selT_t[:], selT[:])
            oneb_t = cpool.tile([128, 1], bf16, tag="oneb")
            nc.sync.dma_start(oneb_t[:], onesb[:])
            gam_t, bet_t, bq_t, bp_t = [], [], [], []
            for t in range(CT):
                g_ = cpool.tile([128, 1], f32, tag=f"gam{t}")
                nc.sync.dma_start(g_[:], gamma[t * 128:(t + 1) * 128, :])
                gam_t.append(g_)
                b_ = cpool.tile([128, 1], f32, tag=f"bet{t}")
                nc.sync.dma_start(b_[:], beta[t * 128:(t + 1) * 128, :])
                bet_t.append(b_)
                q_ = cpool.tile([128, 1], f32, tag=f"bq{t}")
                nc.sync.dma_start(q_[:], bqv[t * 128:(t + 1) * 128, :])
                bq_t.append(q_)
                p_ = cpool.tile([128, 1], f32, tag=f"bp{t}")
                nc.sync.dma_start(p_[:], bpv[t * 128:(t + 1) * 128, :])
                bp_t.append(p_)
            m0_t, wv_t = [], []
            for name, dram, lst in (("m0", m0T, m0_t), ("wv", wvT, wv_t)):
                for t in range(CT):
                    wt = w_p.tile([128, C], bf16, tag=f"{name}{t}")
                    nc.sync.dma_start(wt[:], dram[t * 128:(t + 1) * 128, :])
                    lst.append(wt)
            qkbc_t = []
            for t in range(CT):
                qc = cpool.tile([128, 1], f32, tag=f"qkbc{t}")
                nc.sync.dma_start(qc[:], qkbc[t * 128:(t + 1) * 128, :])
                qkbc_t.append(qc)

            den_acc = acc_p.tile([1, QS], f32, tag="den")
            acc_t = [acc_p.tile([128, QS], f32, tag=f"acc{t}", name=f"acc{t}")
                     for t in range(CT)]

            # ---- GroupNorm stats from bf16 x (bn_stats fused pass) ----
            with tc.tile_pool(name="small", bufs=1) as sm_p, \
                 tc.tile_pool(name="stat_ps", bufs=1, space="PSUM") as stat_ps, \
                 tc.tile_pool(name="ab_ps", bufs=2, space="PSUM") as ab_ps, \
                 tc.tile_pool(name="b_ps", bufs=2, space="PSUM") as b_ps:
                ps_st = stat_ps.tile([8, 8], f32, tag="st")
                for t in range(CT):
                    st = sm_p.tile([128, 8, 6], f32, tag=f"bnst{t}")
                    for g in range(N // 512):
                        nc.vector.bn_stats(st[:, g, :],
                                           xsl(t, g * 512, 512))
                    ag = sm_p.tile([128, 2], f32, tag=f"bnag{t}")
                    nc.vector.bn_aggr(ag[:], st[:])
                    # per-partition mean and E[x^2]
                    s2 = sm_p.tile([128, 1], f32, tag=f"ssq{t}")
                    nc.vector.tensor_tensor(s2[:], ag[:, 0:1], ag[:, 0:1],
                                            alu.mult)
                    nc.vector.tensor_tensor(s2[:], s2[:], ag[:, 1:2], alu.add)
                    nc.tensor.matmul(ps_st[:, t:t + 1], sel_t[:], ag[:, 0:1],
                                     start=True, stop=True)
                    nc.tensor.matmul(ps_st[:, 4 + t:5 + t], sel_t[:], s2[:],
                                     start=True, stop=True)
                st_sb = sm_p.tile([8, 8], f32, tag="st_sb")
                nc.vector.tensor_copy(st_sb[:], ps_st[:])
                mean = sm_p.tile([8, 4], f32, tag="mean")
                nc.vector.tensor_scalar(mean[:], st_sb[:, 0:4],
                                        1.0 / GS, None, op0=alu.mult)
                msq = sm_p.tile([8, 4], f32, tag="msq")
                nc.vector.tensor_scalar(msq[:], st_sb[:, 4:8],
                                        1.0 / GS, None, op0=alu.mult)
                var = sm_p.tile([8, 4], f32, tag="var")
                nc.vector.tensor_tensor(var[:], mean[:], mean[:], alu.mult)
                nc.vector.tensor_tensor(var[:], msq[:], var[:], alu.subtract)
                nc.vector.tensor_scalar(var[:], var[:], EPS, None, op0=alu.add)
                sd = sm_p.tile([8, 4], f32, tag="sd")
                nc.scalar.activation(sd[:], var[:], AF.Sqrt)
                rstd = sm_p.tile([8, 4], f32, tag="rstd")
                nc.vector.reciprocal(rstd[:], sd[:])
                A_t, Bb_t = [], []
                for t in range(CT):
                    ps_ab = ab_ps.tile([128, 2], f32, tag="ab")
                    nc.tensor.matmul(ps_ab[:, 0:1], selT_t[:],
                                     rstd[:, t:t + 1], start=True, stop=True)
                    nc.tensor.matmul(ps_ab[:, 1:2], selT_t[:],
                                     mean[:, t:t + 1], start=True, stop=True)
                    ab = cpool.tile([128, 2], f32, tag=f"ab{t}")
                    nc.vector.tensor_copy(ab[:], ps_ab[:])
                    At = cpool.tile([128, 1], f32, tag=f"A{t}")
                    nc.vector.tensor_tensor(At[:], ab[:, 0:1], gam_t[t][:],
                                            alu.mult)
                    Bt = cpool.tile([128, 1], f32, tag=f"B{t}")
                    nc.vector.tensor_tensor(Bt[:], ab[:, 1:2], At[:], alu.mult)
                    nc.vector.tensor_tensor(Bt[:], bet_t[t][:], Bt[:],
                                            alu.subtract)
                    Bb = cpool.tile([128, 1], bf16, tag=f"Bb{t}")
                    nc.vector.tensor_copy(Bb[:], Bt[:])
                    A_t.append(At)
                    Bb_t.append(Bb)

                # bias terms from RAW weights:
                #   qkb = M0@B + wk^T bq (host const);  Abias = A*qkb
                #   tv  = wv@B  (for the projection-bias fold)
                abias_t, tv_t = [], []
                for co in range(CT):
                    ps_b = b_ps.tile([128, 2], f32, tag="bb")
                    for ci in range(CT):
                        mm(ps_b[:, 0:1],
                           m0_t[ci][:, co * 128:(co + 1) * 128], Bb_t[ci][:],
                           ci == 0, ci == CT - 1)
                    for ci in range(CT):
                        mm(ps_b[:, 1:2],
                           wv_t[ci][:, co * 128:(co + 1) * 128], Bb_t[ci][:],
                           ci == 0, ci == CT - 1)
                    ab2 = cpool.tile([128, 1], f32, tag=f"abias{co}")
                    nc.vector.tensor_tensor(ab2[:], ps_b[:, 0:1],
                                            qkbc_t[co][:], alu.add)
                    nc.vector.tensor_tensor(ab2[:], ab2[:], A_t[co][:],
                                            alu.mult)
                    abias_t.append(ab2)
                    tv = cpool.tile([128, 1], f32, tag=f"tv{co}")
                    nc.vector.tensor_copy(tv[:], ps_b[:, 1:2])
                    tv_t.append(tv)

                # scale in place: m0' rows and wv' rows by A_cin
                for lst in (m0_t, wv_t):
                    for ci in range(CT):
                        nc.vector.tensor_scalar(lst[ci][:], lst[ci][:],
                                                A_t[ci][:], None,
                                                op0=alu.mult)

            # ---- qk projection: qk = A*(M0A @ x) + A*qkb  (fused copy) --
            with tc.tile_pool(name="q_ps", bufs=2, space="PSUM") as q_ps:
                q_t = []
                for co in range(CT):
                    qt = q_p.tile([128, QS], bf16, tag=f"q{co}", name=f"q{co}")
                    for nn in range(QS // 512):
                        ps = q_ps.tile([128, 512], f32, tag="qp")
                        for ci in range(CT):
                            mm(ps[:], m0_t[ci][:, co * 128:(co + 1) * 128],
                               xsl(ci, nn * 512, 512),
                               ci == 0, ci == CT - 1)
                        nc.scalar.activation(qt[:, nn * 512:(nn + 1) * 512],
                                             ps[:], AF.Identity,
                                             bias=abias_t[co][:],
                                             scale=A_t[co][:])
                    q_t.append(qt)

            # ---- projection weights + device bias (overlaps attention) ----
            with tc.tile_pool(name="u_ps", bufs=2, space="PSUM") as u_ps:
                wp_t, wpf_t = [], []
                for t in range(CT):
                    wf = wp_p.tile([128, C], f32, tag=f"wpf{t}")
                    nc.sync.dma_start(wf[:], wpT[t * 128:(t + 1) * 128, :])
                    wpf_t.append(wf)
                    wr = wp_p.tile([128, C], f32r, tag=f"wpr{t}")
                    nc.vector.tensor_copy(wr[:], wf[:])
                    wp_t.append(wr)
                bpd_t = []
                for co in range(CT):
                    ps_u = u_ps.tile([128, 1], f32, tag="u")
                    for ci in range(CT):
                        mm(ps_u[:], wpf_t[ci][:, co * 128:(co + 1) * 128],
                           tv_t[ci][:], ci == 0, ci == CT - 1)
                    bpd = f_p.tile([128, 1], f32, tag=f"bpd{co}")
                    nc.vector.tensor_tensor(bpd[:], ps_u[:], bp_t[co][:],
                                            alu.add)
                    bpd_t.append(bpd)
                xsf_t = []
                for t in range(CT):
                    sf = xs_p.tile([128, QS], f32, tag=f"xsf{t}",
                                   name=f"xsf{t}")
                    nc.sync.dma_start(sf[:], xsf[t * 128:(t + 1) * 128, :])
                    xsf_t.append(sf)

            # ---- attention over key halves ----
            with tc.tile_pool(name="prod_ps", bufs=2, space="PSUM") as prod_ps, \
                 tc.tile_pool(name="sc_ps", bufs=3, space="PSUM") as sc_ps, \
                 tc.tile_pool(name="att_ps", bufs=2, space="PSUM") as att_ps, \
                 tc.tile_pool(name="den_ps", bufs=1, space="PSUM") as den_ps:
                for half in range(NHALF):
                    j0 = half * JQ
                    vT_t = []
                    for jt in range(JT):
                        ps = prod_ps.tile([128, 512], f32, tag="pp")
                        for ci in range(CT):
                            mm(ps[:], xsl(ci, j0 + jt * 128, 128),
                               wv_t[ci][:], ci == 0, ci == CT - 1)
                        vt = vT_p.tile([128, 512], bf16, tag="vT")
                        if jt % 2 == 0:
                            nc.scalar.copy(vt[:], ps[:])
                        else:
                            nc.vector.tensor_copy(vt[:], ps[:])
                        vT_t.append(vt)

                    for ch in range(NCH):
                        i0 = ch * ICH
                        eT = []
                        for jt in range(JT):
                            ps = sc_ps.tile([128, ICH], f32, tag="sc")
                            for ci in range(CT):
                                mm(ps[:], xsl(ci, j0 + jt * 128, 128),
                                   q_t[ci][:, i0:i0 + ICH],
                                   ci == 0, ci == CT - 1)
                            et = e_p.tile([128, ICH], bf16, tag="e")
                            nc.scalar.activation(et[:], ps[:], AF.Exp,
                                                 scale=SCALE)
                            eT.append(et)
                        ps_d = den_ps.tile([1, ICH], f32, tag="den")
                        for jt in range(JT):
                            mm(ps_d[:], oneb_t[:], eT[jt][:],
                               jt == 0, jt == JT - 1)
                        if half == 0:
                            nc.vector.tensor_copy(den_acc[:, i0:i0 + ICH],
                                                  ps_d[:])
                        else:
                            nc.vector.tensor_tensor(den_acc[:, i0:i0 + ICH],
                                                    den_acc[:, i0:i0 + ICH],
                                                    ps_d[:], alu.add)
                        for co in range(CT):
                            ps_a = att_ps.tile([128, ICH], f32, tag="att")
                            for jt in range(JT):
                                mm(ps_a[:],
                                   vT_t[jt][:, co * 128:(co + 1) * 128],
                                   eT[jt][:], jt == 0, jt == JT - 1)
                            if half == 0:
                                nc.vector.tensor_copy(
                                    acc_t[co][:, i0:i0 + ICH], ps_a[:])
                            else:
                                nc.vector.tensor_tensor(
                                    acc_t[co][:, i0:i0 + ICH],
                                    acc_t[co][:, i0:i0 + ICH], ps_a[:],
                                    alu.add)

            # ---- finalize per query chunk (overlaps tail of attention) ----
            with tc.tile_pool(name="outp", bufs=3) as o_p, \
                 tc.tile_pool(name="f_ps", bufs=2, space="PSUM") as f_ps:
                recip = f_p.tile([1, QS], f32, tag="recip")
                rb = f_p.tile([128, QS], f32, tag="rb")
                accn_t = [f_p.tile([128, QS], f32r, tag=f"accn{t}",
                                   name=f"accn{t}") for t in range(CT)]
                for nn in range(QS // 512):
                    sl = slice(nn * 512, (nn + 1) * 512)
                    nc.vector.reciprocal(recip[:, sl], den_acc[:, sl])
                    nc.gpsimd.partition_broadcast(rb[:, sl], recip[:, sl])
                    for t in range(CT):
                        nc.vector.tensor_tensor(accn_t[t][:, sl],
                                                acc_t[t][:, sl], rb[:, sl],
                                                alu.mult)
                    for co in range(CT):
                        ps = f_ps.tile([128, 512], f32, tag="f")
                        for ci in range(CT):
                            mm(ps[:], wp_t[ci][:, co * 128:(co + 1) * 128],
                               accn_t[ci][:, sl], ci == 0, ci == CT - 1)
                        ot = o_p.tile([128, 512], f32, tag="o")
                        nc.vector.scalar_tensor_tensor(
                            ot[:], ps[:], bpd_t[co][:], xsf_t[co][:, sl],
                            op0=alu.add, op1=alu.add)
                        nc.sync.dma_start(
                            out_d[co * 128:(co + 1) * 128, sl], ot[:])

    nc.compile()
    return nc


def kernel(x, gn_gamma, gn_beta, wq, bq, wk, bk, wv, bv, wp, bp):
    import ml_dtypes
    from concourse import bass_utils

    if "nc" not in _CACHE:
        _CACHE["nc"] = _build()
    nc = _CACHE["nc"]

    x = np.asarray(x, np.float32)
    f = np.float32
    bf = ml_dtypes.bfloat16
    wq32 = np.asarray(wq, f)
    wk32 = np.asarray(wk, f)
    m0T = np.ascontiguousarray((wq32.T @ wk32).astype(bf))  # (wk^T wq)^T
    qkbc = (wk32.T @ np.asarray(bq, f)).reshape(C, 1).astype(f)
    wvT = np.ascontiguousarray(np.asarray(wv, f).T.astype(bf))
    wpT = np.ascontiguousarray(np.asarray(wp, f).T)
    bp_eff = (np.asarray(bp, f)
              + np.asarray(wp, f) @ np.asarray(bv, f)).reshape(C, 1)
    sel = np.zeros((128, 8), f)
    for p in range(128):
        sel[p, p // GS] = 1.0
    common = {
        "m0T": m0T, "qkbc": qkbc, "wvT": wvT, "wpT": wpT,
        "gamma": np.asarray(gn_gamma, f).reshape(C, 1),
        "beta": np.asarray(gn_beta, f).reshape(C, 1),
        "bq": np.asarray(bq, f).reshape(C, 1),
        "bp": bp_eff,
        "sel": sel, "selT": np.ascontiguousarray(sel.T),
        "onesb": np.ones((128, 1), bf),
    }
    in_maps = []
    for core in range(NCORES):
        b, s = divmod(core, 4)
        xb = x[b].reshape(C, N)
        # roll so this core's query block occupies columns 0:QS; key order
        # is permuted identically for k and v, and softmax sums are
        # order-invariant, so the program is core-independent.
        xperm = np.ascontiguousarray(np.roll(xb, -s * QS, axis=1))
        in_maps.append({
            **common,
            "xbf": xperm.astype(bf),
            "xsf": np.ascontiguousarray(xb[:, s * QS:(s + 1) * QS]),
        })

    res = bass_utils.run_bass_kernel_spmd(nc, in_maps,
                                          core_ids=list(range(NCORES)))
    _CACHE["last_result"] = res

    out = np.empty((B, C, N), np.float32)
    for core in range(NCORES):
        b, s = divmod(core, 4)
        out[b][:, s * QS:(s + 1) * QS] = res.results[core]["out"]
    return out.reshape(B, C, H, W)



# revision 9
# speedup vs baseline: 1.4193x; 1.4193x over previous
"""AttnBlock v4: fp8 DoubleRow attention core.

Sharding: core = (batch b in {0,1}) x (query slice s in {0..3}, 1024
queries).  Each core redundantly computes full V^T for its batch
(avoids cross-core collectives), attention for its query slice only.
The host rolls x columns per core so the core's query block is always
columns 0:1024 -- identical program, per-core data.

Math: h = GN(x) = A_c * x + B_c per channel (A, B from runtime stats).
  q = (wq*A)@x + (wq@B + bq)     weight columns scaled on device
  k = (wk*A)@x   (k-bias dropped: per-query-constant under softmax)
  v = (wv*A)@x + const; v-bias folded into the projection bias:
      bp_dev = bp + wp@bv + wp@(wv@B).

v4 over v3:
  - scores / attnV / v-production / softmax-denominator matmuls run in
    fp8e4m3 with perf_mode=DoubleRow (2 contraction rows per PE cycle):
    x, q, v, e(=exp scores) quantized to fp8; 256-channel / 256-key
    contraction groups laid out as [128, 2, free] tiles.
  - wv is prescaled by 16 before fp8 quantization (avoids fp8
    subnormals on the ~N(0, 1/512) weights); the 1/16 is folded into
    the host-side wp (wp/16) and the device bias fold (tv*16).
  - final projection in bf16 (wp.T/16 bf16 from host, attn output
    normalized into bf16) instead of f32r.
  - residual add reads the bf16 x tiles already in SBUF (no separate
    f32 x slice DMA).
  - softmax reciprocal hoisted off the finalize critical path (runs
    right after the half-1 denominator for each chunk).
"""

import os
import sys

import numpy as np

for _p in ("/opt/trn_rl_repo", "/root/.axon_site/_ro/trn_rl_repo"):
    if os.path.isdir(_p) and _p not in sys.path:
        sys.path.insert(0, _p)

B, C, H, W = 2, 512, 64, 64
N = H * W
G = 32
GS = C // G
EPS = 1e-6
NCORES = 8
QS = N // 4               # 1024 queries per core
NHALF = 2                 # key halves
JQ = N // NHALF           # 2048 keys per half
JT = JQ // 128            # 16 key tiles per half
KT2 = JT // 2             # 8 DoubleRow key groups per half
ICH = 512                 # query chunk
NCH = QS // ICH           # 2 chunks
CT = C // 128             # 4 channel tiles
NG = 2                    # DoubleRow channel groups (256 ch each)
SCALE = float(C) ** -0.5
WVS = 16.0                # wv prescale into fp8
ESHIFT = -3.0             # exp(s + ESHIFT): keep e in fp8e4m3 range
                          # (max scaled score ~7.2; e4m3 max normal 240);
                          # a constant shift cancels in the softmax ratio

_CACHE = {}


def _build():
    import contextlib

    import concourse.mybir as mybir
    import concourse.tile as tile
    from concourse import bacc
    from concourse.alu_op_type import AluOpType as alu

    f32 = mybir.dt.float32
    bf16 = mybir.dt.bfloat16
    f8 = mybir.dt.float8e4
    AF = mybir.ActivationFunctionType
    PM = mybir.MatmulPerfMode

    nc = bacc.Bacc("TRN2", target_bir_lowering=False, debug=False,
                   num_devices=NCORES)

    xbf = nc.dram_tensor("xbf", [C, N], bf16, kind="ExternalInput").ap()
    x8d = [nc.dram_tensor(f"x8_{g}", [128, 2, N], f8,
                          kind="ExternalInput").ap() for g in range(NG)]
    m0T = nc.dram_tensor("m0T", [C, C], bf16, kind="ExternalInput").ap()
    qkbc = nc.dram_tensor("qkbc", [C, 1], f32, kind="ExternalInput").ap()
    wvT = nc.dram_tensor("wvT", [C, C], bf16, kind="ExternalInput").ap()
    wpT = nc.dram_tensor("wpT", [C, C], bf16, kind="ExternalInput").ap()
    gamma = nc.dram_tensor("gamma", [C, 1], f32, kind="ExternalInput").ap()
    beta = nc.dram_tensor("beta", [C, 1], f32, kind="ExternalInput").ap()
    bqv = nc.dram_tensor("bq", [C, 1], f32, kind="ExternalInput").ap()
    bpv = nc.dram_tensor("bp", [C, 1], f32, kind="ExternalInput").ap()
    sel = nc.dram_tensor("sel", [128, 8], f32, kind="ExternalInput").ap()
    selT = nc.dram_tensor("selT", [8, 128], f32, kind="ExternalInput").ap()
    ones8 = nc.dram_tensor("ones8", [128, 2, 128], f8,
                           kind="ExternalInput").ap()
    out_d = nc.dram_tensor("out", [C, QS], f32, kind="ExternalOutput").ap()

    def mm(ps, lhsT, rhs, start, stop):
        nc.tensor.matmul(ps, lhsT, rhs, start=start, stop=stop)

    def mm8(ps, lhsT, rhs, start, stop):
        nc.tensor.matmul(ps, lhsT, rhs, start=start, stop=stop,
                         perf_mode=PM.DoubleRow)

    with tile.TileContext(nc) as tc:
        outer = contextlib.ExitStack()
        with outer:
            cpool = outer.enter_context(tc.tile_pool(name="const", bufs=1))
            x_p = outer.enter_context(tc.tile_pool(name="xbf", bufs=1))
            x8_p = outer.enter_context(tc.tile_pool(name="x8", bufs=1))
            acc_p = outer.enter_context(tc.tile_pool(name="acc", bufs=1))
            w_p = outer.enter_context(tc.tile_pool(name="wts", bufs=1))
            q8_p = outer.enter_context(tc.tile_pool(name="q8", bufs=1))
            v8_p = outer.enter_context(tc.tile_pool(name="v8", bufs=KT2))
            e8_p = outer.enter_context(tc.tile_pool(name="e8", bufs=KT2 + 2))
            wp_p = outer.enter_context(tc.tile_pool(name="wp", bufs=1))
            f_p = outer.enter_context(tc.tile_pool(name="fin", bufs=1))

            # ---- x first (stats critical path), then consts/weights ----
            x_t = []
            for t in range(CT):
                row = []
                for c in range(N // 1024):
                    xt = x_p.tile([128, 1024], bf16, tag=f"x{t}_{c}",
                                  name=f"x{t}_{c}")
                    nc.sync.dma_start(
                        xt[:], xbf[t * 128:(t + 1) * 128,
                                   c * 1024:(c + 1) * 1024])
                    row.append(xt)
                x_t.append(row)

            def xsl(ci, start, size):
                c, off = divmod(start, 1024)
                assert off + size <= 1024
                return x_t[ci][c][:, off:off + size]

            sel_t = cpool.tile([128, 8], f32, tag="sel")
            nc.sync.dma_start(sel_t[:], sel[:])
            selT_t = cpool.tile([8, 128], f32, tag="selT")
            nc.sync.dma_start(selT_t[:], selT[:])
            ones8_t = cpool.tile([128, 2, 128], f8, tag="ones8")
            nc.sync.dma_start(ones8_t[:], ones8[:])
            esh_t = cpool.tile([128, 1], f32, tag="esh")
            nc.vector.memset(esh_t[:], ESHIFT)
            gam_t, bet_t, bq_t, bp_t = [], [], [], []
            for t in range(CT):
                g_ = cpool.tile([128, 1], f32, tag=f"gam{t}")
                nc.sync.dma_start(g_[:], gamma[t * 128:(t + 1) * 128, :])
                gam_t.append(g_)
                b_ = cpool.tile([128, 1], f32, tag=f"bet{t}")
                nc.sync.dma_start(b_[:], beta[t * 128:(t + 1) * 128, :])
                bet_t.append(b_)
                q_ = cpool.tile([128, 1], f32, tag=f"bq{t}")
                nc.sync.dma_start(q_[:], bqv[t * 128:(t + 1) * 128, :])
                bq_t.append(q_)
                p_ = cpool.tile([128, 1], f32, tag=f"bp{t}")
                nc.sync.dma_start(p_[:], bpv[t * 128:(t + 1) * 128, :])
                bp_t.append(p_)
            m0_t, wv_t = [], []
            for name, dram, lst in (("m0", m0T, m0_t), ("wv", wvT, wv_t)):
                for t in range(CT):
                    wt = w_p.tile([128, C], bf16, tag=f"{name}{t}")
                    nc.sync.dma_start(wt[:], dram[t * 128:(t + 1) * 128, :])
                    lst.append(wt)
            x8_t = []
            for g in range(NG):
                xt8 = x8_p.tile([128, 2, N], f8, tag=f"x8_{g}",
                                name=f"x8_{g}")
                nc.sync.dma_start(xt8[:], x8d[g][:])
                x8_t.append(xt8)
            qkbc_t = []
            for t in range(CT):
                qc = cpool.tile([128, 1], f32, tag=f"qkbc{t}")
                nc.sync.dma_start(qc[:], qkbc[t * 128:(t + 1) * 128, :])
                qkbc_t.append(qc)

            den_acc = acc_p.tile([1, QS], f32, tag="den")
            recip = acc_p.tile([1, QS], f32, tag="recip")
            acc_t = [acc_p.tile([128, QS], f32, tag=f"acc{t}", name=f"acc{t}")
                     for t in range(CT)]

            # ---- GroupNorm stats from bf16 x (bn_stats fused pass) ----
            with tc.tile_pool(name="small", bufs=1) as sm_p, \
                 tc.tile_pool(name="stat_ps", bufs=1, space="PSUM") as stat_ps, \
                 tc.tile_pool(name="ab_ps", bufs=2, space="PSUM") as ab_ps, \
                 tc.tile_pool(name="b_ps", bufs=2, space="PSUM") as b_ps:
                ps_st = stat_ps.tile([8, 8], f32, tag="st")
                for t in range(CT):
                    st = sm_p.tile([128, 8, 6], f32, tag=f"bnst{t}")
                    for g in range(N // 512):
                        nc.vector.bn_stats(st[:, g, :],
                                           xsl(t, g * 512, 512))
                    ag = sm_p.tile([128, 2], f32, tag=f"bnag{t}")
                    nc.vector.bn_aggr(ag[:], st[:])
                    # per-partition mean and E[x^2]
                    s2 = sm_p.tile([128, 1], f32, tag=f"ssq{t}")
                    nc.vector.tensor_tensor(s2[:], ag[:, 0:1], ag[:, 0:1],
                                            alu.mult)
                    nc.vector.tensor_tensor(s2[:], s2[:], ag[:, 1:2], alu.add)
                    nc.tensor.matmul(ps_st[:, t:t + 1], sel_t[:], ag[:, 0:1],
                                     start=True, stop=True)
                    nc.tensor.matmul(ps_st[:, 4 + t:5 + t], sel_t[:], s2[:],
                                     start=True, stop=True)
                st_sb = sm_p.tile([8, 8], f32, tag="st_sb")
                nc.vector.tensor_copy(st_sb[:], ps_st[:])
                mean = sm_p.tile([8, 4], f32, tag="mean")
                nc.vector.tensor_scalar(mean[:], st_sb[:, 0:4],
                                        1.0 / GS, None, op0=alu.mult)
                msq = sm_p.tile([8, 4], f32, tag="msq")
                nc.vector.tensor_scalar(msq[:], st_sb[:, 4:8],
                                        1.0 / GS, None, op0=alu.mult)
                var = sm_p.tile([8, 4], f32, tag="var")
                nc.vector.tensor_tensor(var[:], mean[:], mean[:], alu.mult)
                nc.vector.tensor_tensor(var[:], msq[:], var[:], alu.subtract)
                nc.vector.tensor_scalar(var[:], var[:], EPS, None, op0=alu.add)
                sd = sm_p.tile([8, 4], f32, tag="sd")
                nc.scalar.activation(sd[:], var[:], AF.Sqrt)
                rstd = sm_p.tile([8, 4], f32, tag="rstd")
                nc.vector.reciprocal(rstd[:], sd[:])
                A_t, A16_t, Bb_t = [], [], []
                for t in range(CT):
                    ps_ab = ab_ps.tile([128, 2], f32, tag="ab")
                    nc.tensor.matmul(ps_ab[:, 0:1], selT_t[:],
                                     rstd[:, t:t + 1], start=True, stop=True)
                    nc.tensor.matmul(ps_ab[:, 1:2], selT_t[:],
                                     mean[:, t:t + 1], start=True, stop=True)
                    ab = cpool.tile([128, 2], f32, tag=f"ab{t}")
                    nc.vector.tensor_copy(ab[:], ps_ab[:])
                    At = cpool.tile([128, 1], f32, tag=f"A{t}")
                    nc.vector.tensor_tensor(At[:], ab[:, 0:1], gam_t[t][:],
                                            alu.mult)
                    At16 = cpool.tile([128, 1], f32, tag=f"A16_{t}")
                    nc.vector.tensor_scalar(At16[:], At[:], WVS, None,
                                            op0=alu.mult)
                    Bt = cpool.tile([128, 1], f32, tag=f"B{t}")
                    nc.vector.tensor_tensor(Bt[:], ab[:, 1:2], At[:], alu.mult)
                    nc.vector.tensor_tensor(Bt[:], bet_t[t][:], Bt[:],
                                            alu.subtract)
                    Bb = cpool.tile([128, 1], bf16, tag=f"Bb{t}")
                    nc.vector.tensor_copy(Bb[:], Bt[:])
                    A_t.append(At)
                    A16_t.append(At16)
                    Bb_t.append(Bb)

                # bias terms from RAW weights:
                #   qkb = M0@B + wk^T bq (host const);  Abias = A*qkb
                #   tv  = wv@B  (for the projection-bias fold)
                abias_t, tvb_t = [], []
                for co in range(CT):
                    ps_b = b_ps.tile([128, 2], f32, tag="bb")
                    for ci in range(CT):
                        mm(ps_b[:, 0:1],
                           m0_t[ci][:, co * 128:(co + 1) * 128], Bb_t[ci][:],
                           ci == 0, ci == CT - 1)
                    for ci in range(CT):
                        mm(ps_b[:, 1:2],
                           wv_t[ci][:, co * 128:(co + 1) * 128], Bb_t[ci][:],
                           ci == 0, ci == CT - 1)
                    ab2 = cpool.tile([128, 1], f32, tag=f"abias{co}")
                    nc.vector.tensor_tensor(ab2[:], ps_b[:, 0:1],
                                            qkbc_t[co][:], alu.add)
                    nc.vector.tensor_tensor(ab2[:], ab2[:], A_t[co][:],
                                            alu.mult)
                    abias_t.append(ab2)
                    tvb = cpool.tile([128, 1], bf16, tag=f"tvb{co}")
                    nc.vector.tensor_copy(tvb[:], ps_b[:, 1:2])
                    tvb_t.append(tvb)

                # scale m0 rows by A_cin in place (bf16, q-projection);
                # wv stays raw -- the fp8 conversion below applies A*16.
                for ci in range(CT):
                    nc.vector.tensor_scalar(m0_t[ci][:], m0_t[ci][:],
                                            A_t[ci][:], None, op0=alu.mult)

                # wv8[g][:, i, :] = fp8(A16 * wv_raw rows), t = 2g + i
                wv8_t = []
                for g in range(NG):
                    w8 = w_p.tile([128, 2, C], f8, tag=f"wv8_{g}")
                    for i in range(2):
                        t = 2 * g + i
                        nc.scalar.activation(w8[:, i, :], wv_t[t][:],
                                             AF.Identity, scale=A16_t[t][:])
                    wv8_t.append(w8)

            # ---- qk projection -> fp8 DR tiles: q8[g][:, i, :] ----
            with tc.tile_pool(name="q_ps", bufs=2, space="PSUM") as q_ps:
                q8_t = [q8_p.tile([128, 2, QS], f8, tag=f"q8_{g}",
                                  name=f"q8_{g}") for g in range(NG)]
                for co in range(CT):
                    g, i = divmod(co, 2)
                    for nn in range(QS // 512):
                        ps = q_ps.tile([128, 512], f32, tag="qp")
                        for ci in range(CT):
                            mm(ps[:], m0_t[ci][:, co * 128:(co + 1) * 128],
                               xsl(ci, nn * 512, 512),
                               ci == 0, ci == CT - 1)
                        nc.scalar.activation(
                            q8_t[g][:, i, nn * 512:(nn + 1) * 512],
                            ps[:], AF.Identity,
                            bias=abias_t[co][:], scale=A_t[co][:])

            # ---- projection weights (bf16, host wp.T/16) + device bias ----
            with tc.tile_pool(name="u_ps", bufs=2, space="PSUM") as u_ps:
                wp_t = []
                for t in range(CT):
                    wr = wp_p.tile([128, C], bf16, tag=f"wp{t}")
                    nc.sync.dma_start(wr[:], wpT[t * 128:(t + 1) * 128, :])
                    wp_t.append(wr)
                bpd_t = []
                for co in range(CT):
                    ps_u = u_ps.tile([128, 1], f32, tag="u")
                    for ci in range(CT):
                        mm(ps_u[:], wp_t[ci][:, co * 128:(co + 1) * 128],
                           tvb_t[ci][:], ci == 0, ci == CT - 1)
                    bpd = f_p.tile([128, 1], f32, tag=f"bpd{co}")
                    # wp tiles carry wp/16: restore with *16, then + bp
                    nc.vector.scalar_tensor_tensor(
                        bpd[:], ps_u[:], WVS, bp_t[co][:],
                        op0=alu.mult, op1=alu.add)
                    bpd_t.append(bpd)

            # ---- attention over key halves (fp8 DoubleRow) ----
            with tc.tile_pool(name="mm_ps", bufs=3, space="PSUM") as mm_ps, \
                 tc.tile_pool(name="att_ps", bufs=2, space="PSUM") as att_ps, \
                 tc.tile_pool(name="den_ps", bufs=1, space="PSUM") as den_ps:
                for half in range(NHALF):
                    j0 = half * JQ
                    v8_t = []
                    for jt in range(JT):
                        ps = mm_ps.tile([128, 512], f32, tag="mm")
                        for g in range(NG):
                            mm8(ps[:],
                                x8_t[g][:, :, j0 + jt * 128:
                                        j0 + (jt + 1) * 128],
                                wv8_t[g][:], g == 0, g == NG - 1)
                        kt2, slot = divmod(jt, 2)
                        if slot == 0:
                            vt = v8_p.tile([128, 2, C], f8, tag="v8")
                            v8_t.append(vt)
                        if jt % 4 < 2:
                            nc.scalar.copy(v8_t[kt2][:, slot, :], ps[:])
                        else:
                            nc.vector.tensor_copy(v8_t[kt2][:, slot, :],
                                                  ps[:])

                    for ch in range(NCH):
                        i0 = ch * ICH
                        e8_t = []
                        for jt in range(JT):
                            ps = mm_ps.tile([128, ICH], f32, tag="mm")
                            for g in range(NG):
                                mm8(ps[:],
                                    x8_t[g][:, :, j0 + jt * 128:
                                            j0 + (jt + 1) * 128],
                                    q8_t[g][:, :, i0:i0 + ICH],
                                    g == 0, g == NG - 1)
                            kt2, slot = divmod(jt, 2)
                            if slot == 0:
                                et = e8_p.tile([128, 2, ICH], f8, tag="e8")
                                e8_t.append(et)
                            nc.scalar.activation(e8_t[kt2][:, slot, :],
                                                 ps[:], AF.Exp, scale=SCALE,
                                                 bias=esh_t[:])
                        # denominator: all-ones stationary (every output
                        # partition carries the same key-sum row; row 0 used)
                        ps_d = den_ps.tile([128, ICH], f32, tag="den")
                        for kt2 in range(KT2):
                            mm8(ps_d[:], ones8_t[:], e8_t[kt2][:],
                                kt2 == 0, kt2 == KT2 - 1)
                        if half == 0:
                            nc.vector.tensor_copy(den_acc[:, i0:i0 + ICH],
                                                  ps_d[0:1, :])
                        else:
                            nc.vector.tensor_tensor(den_acc[:, i0:i0 + ICH],
                                                    den_acc[:, i0:i0 + ICH],
                                                    ps_d[0:1, :], alu.add)
                            nc.vector.reciprocal(recip[:, i0:i0 + ICH],
                                                 den_acc[:, i0:i0 + ICH])
                        for co in range(CT):
                            ps_a = att_ps.tile([128, ICH], f32, tag="att")
                            for kt2 in range(KT2):
                                mm8(ps_a[:],
                                    v8_t[kt2][:, :, co * 128:(co + 1) * 128],
                                    e8_t[kt2][:], kt2 == 0, kt2 == KT2 - 1)
                            if half == 0:
                                nc.vector.tensor_copy(
                                    acc_t[co][:, i0:i0 + ICH], ps_a[:])
                            else:
                                nc.vector.tensor_tensor(
                                    acc_t[co][:, i0:i0 + ICH],
                                    acc_t[co][:, i0:i0 + ICH], ps_a[:],
                                    alu.add)

            # ---- finalize per query chunk (overlaps tail of attention) ----
            with tc.tile_pool(name="outp", bufs=3) as o_p, \
                 tc.tile_pool(name="f_ps", bufs=2, space="PSUM") as f_ps:
                rb = f_p.tile([128, QS], f32, tag="rb")
                accn_t = [f_p.tile([128, QS], bf16, tag=f"accn{t}",
                                   name=f"accn{t}") for t in range(CT)]
                for nn in range(QS // 512):
                    sl = slice(nn * 512, (nn + 1) * 512)
                    nc.gpsimd.partition_broadcast(rb[:, sl], recip[:, sl])
                    for t in range(CT):
                        nc.vector.tensor_tensor(accn_t[t][:, sl],
                                                acc_t[t][:, sl], rb[:, sl],
                                                alu.mult)
                    for co in range(CT):
                        ps = f_ps.tile([128, 512], f32, tag="f")
                        for ci in range(CT):
                            mm(ps[:], wp_t[ci][:, co * 128:(co + 1) * 128],
                               accn_t[ci][:, sl], ci == 0, ci == CT - 1)
                        ot = o_p.tile([128, 512], f32, tag="o")
                        nc.vector.scalar_tensor_tensor(
                            ot[:], ps[:], bpd_t[co][:], x_t[co][0][:, sl],
                            op0=alu.add, op1=alu.add)
                        nc.sync.dma_start(
                            out_d[co * 128:(co + 1) * 128, sl], ot[:])

    nc.compile()
    return nc


def kernel(x, gn_gamma, gn_beta, wq, bq, wk, bk, wv, bv, wp, bp):
    import ml_dtypes
    from concourse import bass_utils

    if "nc" not in _CACHE:
        _CACHE["nc"] = _build()
    nc = _CACHE["nc"]

    x = np.asarray(x, np.float32)
    f = np.float32
    bf = ml_dtypes.bfloat16
    f8 = ml_dtypes.float8_e4m3
    wq32 = np.asarray(wq, f)
    wk32 = np.asarray(wk, f)
    m0T = np.ascontiguousarray((wq32.T @ wk32).astype(bf))  # (wk^T wq)^T
    qkbc = (wk32.T @ np.asarray(bq, f)).reshape(C, 1).astype(f)
    wvT = np.ascontiguousarray(np.asarray(wv, f).T.astype(bf))
    wpT = np.ascontiguousarray((np.asarray(wp, f).T / WVS).astype(bf))
    bp_eff = (np.asarray(bp, f)
              + np.asarray(wp, f) @ np.asarray(bv, f)).reshape(C, 1)
    sel = np.zeros((128, 8), f)
    for p in range(128):
        sel[p, p // GS] = 1.0
    common = {
        "m0T": m0T, "qkbc": qkbc, "wvT": wvT, "wpT": wpT,
        "gamma": np.asarray(gn_gamma, f).reshape(C, 1),
        "beta": np.asarray(gn_beta, f).reshape(C, 1),
        "bq": np.asarray(bq, f).reshape(C, 1),
        "bp": bp_eff,
        "sel": sel, "selT": np.ascontiguousarray(sel.T),
        "ones8": np.ones((128, 2, 128), f8),
    }
    in_maps = []
    for core in range(NCORES):
        b, s = divmod(core, 4)
        xb = x[b].reshape(C, N)
        # roll so this core's query block occupies columns 0:QS; key order
        # is permuted identically for all key-side tensors, and softmax
        # sums are order-invariant, so the program is core-independent.
        xperm = np.ascontiguousarray(np.roll(xb, -s * QS, axis=1))
        im = {**common, "xbf": xperm.astype(bf)}
        for g in range(NG):
            x8g = xperm[g * 256:(g + 1) * 256].reshape(2, 128, N)
            im[f"x8_{g}"] = np.ascontiguousarray(
                x8g.transpose(1, 0, 2)).astype(f8)
        in_maps.append(im)

    res = bass_utils.run_bass_kernel_spmd(nc, in_maps,
                                          core_ids=list(range(NCORES)))
    _CACHE["last_result"] = res

    out = np.empty((B, C, N), np.float32)
    for core in range(NCORES):
        b, s = divmod(core, 4)
        out[b][:, s * QS:(s + 1) * QS] = res.results[core]["out"]
    return out.reshape(B, C, H, W)
